# revision 13
# baseline (speedup 1.0000x reference)
"""CapsuleLayer (B=32, J=32, I=2048, T=16, D=16, 3 routing iters) on 8 TRN2 cores.

Shard input-capsule axis I across 8 cores (I_loc=256). W cast to bf16 on host
(DMA 4.2MB/core). Per core:
  - s0 computed DIRECTLY from W via dense x stationary (K=(i8,d)=128) while W
    streams in; AllReduce(s0) overlaps phase A -> v0 ready early.
  - phase A: u_hat via block-diag x stationaries (K=64); PSUM->SBUF copies
    split ACT/Pool 50:50 so the copy drain keeps pace with the PE.
  - routing iterations with 16-g chunks; b2 = u.(v0+v1) (fresh logits, no
    carried bij); p2 emitted in halves so the ones-restream trails closely.
  - AllReduce in bf16 with a tiny warmup collective at t=0; AR staging DMAs
    ride the ACT/DVE queues so the W stream on sync is never blocked.
  - squash computed redundantly on all 128 partitions (no vexp broadcast).
"""

import functools
import sys

import numpy as np

sys.path.insert(0, "/opt/trn_rl_repo")

import ml_dtypes  # noqa: E402

import concourse.bacc as bacc  # noqa: E402
import concourse.mybir as mybir  # noqa: E402
import concourse.tile as tile  # noqa: E402

F32 = mybir.dt.float32
BF16 = mybir.dt.bfloat16

NCORES = 8
B, J, I, T, D = 32, 32, 2048, 16, 16
ILOC = I // NCORES          # 256
G = ILOC // 4               # 64 groups of 4 i's
NWT = G // 2                # 32 W tiles [128, 512], tile p holds g=2p, 2p+1
NWG = 8                     # W DMA groups of 4 tiles
NCH = 4                     # routing chunks
CG = G // NCH               # 16 g per chunk
H = CG // 2                 # half-chunk groups
EPS = 1e-9
AluOp = mybir.AluOpType
Act = mybir.ActivationFunctionType


def _build_program(single=False):
    nc = bacc.Bacc(
        "TRN2",
        target_bir_lowering=False,
        debug=False,
        enable_asserts=False,
        num_devices=1 if single else NCORES,
    )

    wt_d = nc.dram_tensor("wt", [NWG, 128, 4 * 512], BF16, kind="ExternalInput")
    xall_d = nc.dram_tensor("xall", [128, NWT * 128], BF16, kind="ExternalInput")
    xs8_d = nc.dram_tensor("xs8", [128, NWT * 32], BF16, kind="ExternalInput")
    ones_d = nc.dram_tensor("onesdb", [128, 32], BF16, kind="ExternalInput")
    out_d = nc.dram_tensor("outv", [32, 512], F32, kind="ExternalOutput")

    with tile.TileContext(nc) as tc:
        _capsule(tc, wt_d.ap(), xall_d.ap(), xs8_d.ap(), ones_d.ap(), out_d.ap(),
                 single=single)
    nc.compile()
    return nc


def _capsule(tc, wt, xall_dr, xs8_dr, ones_dr, outv, single=False):
    nc = tc.nc
    from contextlib import ExitStack

    ctx = ExitStack()
    with ctx:
        up = ctx.enter_context(tc.tile_pool(name="u", bufs=1))
        wp = ctx.enter_context(tc.tile_pool(name="w", bufs=3))
        cp = ctx.enter_context(tc.tile_pool(name="consts", bufs=1))
        qp = ctx.enter_context(tc.tile_pool(name="q", bufs=1))
        pp = ctx.enter_context(tc.tile_pool(name="p2", bufs=3))
        tp = ctx.enter_context(tc.tile_pool(name="tree", bufs=1))
        bp = ctx.enter_context(tc.tile_pool(name="bij", bufs=2))
        sp = ctx.enter_context(tc.tile_pool(name="small", bufs=2))
        pA = ctx.enter_context(tc.tile_pool(name="psA", bufs=4, space="PSUM"))
        pS0 = ctx.enter_context(tc.tile_pool(name="psS0", bufs=1, space="PSUM"))
        pSS = ctx.enter_context(tc.tile_pool(name="psSS", bufs=2, space="PSUM"))
        dp = ctx.enter_context(tc.tile_pool(name="dram", bufs=8, space="DRAM"))

        # ---- persistent SBUF tiles
        xall = cp.tile([128, NWT * 128], BF16)
        xs8 = cp.tile([128, NWT * 32], BF16)
        ones = cp.tile([128, 32], BF16)
        uch = [up.tile([128, CG * 512], BF16, tag=f"u{c}", name=f"u{c}")
               for c in range(NCH)]

        nc.sync.dma_start(ones[:, :], ones_dr)
        nc.sync.dma_start(xs8[:, :], xs8_dr)
        nc.sync.dma_start(xall[:, :], xall_dr)

        # ---- warmup collective (absorb CC cold-start; overlaps W DMA)
        dumz = sp.tile([32, 16], F32, tag="dumz")
        nc.vector.memset(dumz[:, :], 0.0)
        dum_in = dp.tile([32, 16], F32, tag="dumin")
        dum_out = dp.tile([32, 16], F32, tag="dumout")
        nc.sync.dma_start(dum_in[:, :], dumz[:, :])
        if not single:
            nc.gpsimd.collective_compute(
                "AllReduce", AluOp.add,
                replica_groups=[list(range(NCORES))],
                ins=[dum_in[:, :].opt()], outs=[dum_out[:, :].opt()],
            )

        # ---- W DMA pass 1 + s0-direct matmuls (dense K=128 stationaries)
        ps0 = pS0.tile([32, 512], F32, tag="s0")
        for g in range(NWG):
            w = wp.tile([128, 4 * 512], BF16, tag="w")
            nc.sync.dma_start(w[:, :], wt[g])
            for k in range(4):
                p = 4 * g + k
                nc.tensor.matmul(
                    ps0[:, :],
                    lhsT=xs8[:, p * 32:(p + 1) * 32],
                    rhs=w[:, k * 512:(k + 1) * 512],
                    start=(p == 0), stop=(p == NWT - 1),
                )
        # psum free = (j,t) -> s layout (t,j), scaled by 1/J, cast bf16
        s0bf = sp.tile([32, 512], BF16, tag="s_bf")
        nc.scalar.mul(
            s0bf[:, :].rearrange("p (t j) -> p t j", t=16),
            ps0[:, :].rearrange("p (j t) -> p t j", j=32),
            1.0 / J,
        )
        ccout0 = _ar_launch(tc, dp, s0bf, r=0, single=single)

        # ---- phase A (u_hat). Part 1 (chunks 0,1) now; part 2 (chunks 2,3)
        # is deferred into the iter-1 emission so its ACT copies slot into
        # the gaps between iter-1 exp ops instead of blocking them.
        def phaseA_group(g):
            w = wp.tile([128, 4 * 512], BF16, tag="w")
            nc.sync.dma_start(w[:, :], wt[g])
            for k in range(4):
                p = 4 * g + k
                for gl in range(2):
                    gg = 2 * p + gl          # group index 0..63
                    ch, gc = gg // CG, gg % CG
                    ps = pA.tile([128, 512], F32, tag="pA")
                    nc.tensor.matmul(
                        ps[:, :],
                        lhsT=xall[gl * 64:(gl + 1) * 64, p * 128:(p + 1) * 128],
                        rhs=w[gl * 64:(gl + 1) * 64, k * 512:(k + 1) * 512],
                        start=True, stop=True,
                    )
                    # psum free=(j,t); u free=(t,j)
                    src = ps[:, :].rearrange("p (j t) -> p t j", j=32)
                    dst = uch[ch][:, gc * 512:(gc + 1) * 512].rearrange(
                        "p (t j) -> p t j", t=16)
                    if ch == 0 and gc % 2 == 0:
                        nc.vector.tensor_copy(dst, src)
                    else:
                        nc.scalar.copy(dst, src)

        for g in range(4):
            phaseA_group(g)

        # ---- routing iterations
        vexp = _squash(tc, sp, ccout0, r=0)   # [128,512] bf16 v-broadcast
        for r in (1, 2):
            psum_s = pSS.tile([32, 512], F32, tag="sacc")
            for ch in range(NCH):
                usl = uch[ch][:, :].rearrange("p (g t j) -> p g t j", g=CG, t=16)
                # q = u * vhat  [128, CG*512]
                q = qp.tile([128, CG * 512], BF16, tag="q")
                vb = (vexp[:, :].rearrange("p (t j) -> p t j", t=16)
                      .unsqueeze(1).to_broadcast([128, CG, 16, 32]))
                nc.vector.tensor_mul(
                    q[:, :].rearrange("p (g t j) -> p g t j", g=CG, t=16), usl, vb)
                # tree-reduce over t
                q4 = q[:, :].rearrange("p (g t j) -> p g t j", g=CG, t=16)
                l1 = tp.tile([128, CG * 256], BF16, tag="l1")
                l14 = l1[:, :].rearrange("p (g t j) -> p g t j", g=CG, t=8)
                nc.vector.tensor_add(l14, q4[:, :, 0:8, :], q4[:, :, 8:16, :])
                l2 = tp.tile([128, CG * 128], BF16, tag="l2")
                l24 = l2[:, :].rearrange("p (g t j) -> p g t j", g=CG, t=4)
                nc.vector.tensor_add(l24, l14[:, :, 0:4, :], l14[:, :, 4:8, :])
                l3 = tp.tile([128, CG * 64], BF16, tag="l3")
                l34 = l3[:, :].rearrange("p (g t j) -> p g t j", g=CG, t=2)
                nc.vector.tensor_add(l34, l24[:, :, 0:2, :], l24[:, :, 2:4, :])
                bij = bp.tile([128, CG * 32], F32, tag="bij")
                nc.vector.tensor_add(
                    bij[:, :].rearrange("p (g j) -> p g j", g=CG),
                    l34[:, :, 0, :], l34[:, :, 1, :])
                # softmax over j
                cte = bp.tile([128, CG * 32], BF16, tag="cte")
                nc.scalar.activation(cte[:, :], bij[:, :], Act.Exp)
                z = sp.tile([128, CG], F32, tag="z")
                nc.vector.tensor_reduce(
                    z[:, :], cte[:, :].rearrange("p (g j) -> p g j", g=CG),
                    mybir.AxisListType.X, AluOp.add)
                invz = sp.tile([128, CG], F32, tag="invz")
                nc.vector.reciprocal(invz[:, :], z[:, :])
                invzb = sp.tile([128, CG], BF16, tag="invzb")
                nc.vector.tensor_copy(invzb[:, :], invz[:, :])
                cc = bp.tile([128, CG * 32], BF16, tag="cc")
                nc.vector.tensor_mul(
                    cc[:, :].rearrange("p (g j) -> p g j", g=CG),
                    cte[:, :].rearrange("p (g j) -> p g j", g=CG),
                    invzb[:, :].unsqueeze(2).to_broadcast([128, CG, 32]))
                # p2 = u * c in two half-tiles; restream trails each half
                ccb = (cc[:, :].rearrange("p (g j) -> p g j", g=CG)
                       .unsqueeze(2).to_broadcast([128, CG, 16, 32]))
                for h in range(2):
                    gs = slice(h * H, (h + 1) * H)
                    p2 = pp.tile([128, H * 512], BF16, tag="p2")
                    nc.vector.tensor_mul(
                        p2[:, :].rearrange("p (g t j) -> p g t j", g=H, t=16),
                        usl[:, gs], ccb[:, gs])
                    for gl in range(H):
                        gg = ch * CG + h * H + gl
                        nc.tensor.matmul(
                            psum_s[:, :],
                            lhsT=ones[:, :],
                            rhs=p2[:, gl * 512:(gl + 1) * 512],
                            start=(gg == 0), stop=(gg == G - 1),
                        )
                # deferred phase-A part 2: chunk-2 groups after iter-1 chunk
                # 0, chunk-3 groups after iter-1 chunk 1
                if r == 1 and ch < 2:
                    phaseA_group(4 + 2 * ch)
                    phaseA_group(5 + 2 * ch)
            sbf = sp.tile([32, 512], BF16, tag="s_bf")
            nc.scalar.copy(sbf[:, :], psum_s[:, :])
            ccout = _ar_launch(tc, dp, sbf, r=r, single=single)
            vexp_new = _squash(tc, sp, ccout, r=r,
                               out_f32=(outv if r == 2 else None))
            if r == 1:
                # iter2 logits use b2 = u.(v0+v1)
                vsum = sp.tile([128, 512], BF16, tag="vsum")
                nc.vector.tensor_add(vsum[:, :], vexp[:, :], vexp_new[:, :])
                vexp = vsum


def _ar_launch(tc, dp, s_bf, r, single=False):
    """Stage s to DRAM (via ACT queue) and trigger the bf16 AllReduce."""
    nc = tc.nc
    ccin = dp.tile([32, 512], BF16, tag=f"ccin{r}")
    ccout = dp.tile([32, 512], BF16, tag=f"ccout{r}")
    nc.scalar.dma_start(ccin[:, :], s_bf[:, :])
    if single:
        nc.scalar.dma_start(ccout[:, :], ccin[:, :])
    else:
        nc.gpsimd.collective_compute(
            "AllReduce", AluOp.add,
            replica_groups=[list(range(NCORES))],
            ins=[ccin[:, :].opt()], outs=[ccout[:, :].opt()],
        )
    return ccout


def _squash(tc, sp, ccout, r, out_f32=None):
    """Replicate AR result to 128 partitions (sync queue), then squash.

    Returns vexp [128,512] bf16 = v replicated to the (i_sub,b) layout.
    If out_f32 is given (final iter), also DMA v rows 0:32 as f32 to it.
    """
    nc = tc.nc
    s128 = sp.tile([128, 512], BF16, tag=f"s128_{r}")
    for k in range(4):
        nc.gpsimd.dma_start(s128[k * 32:(k + 1) * 32, :], ccout[:, :])

    # squash: v = s * |s|^2/(1+|s|^2)/sqrt(|s|^2+eps), |.| over t
    sq = sp.tile([128, 512], BF16, tag="sq")
    nc.vector.tensor_mul(sq[:, :], s128[:, :], s128[:, :])
    ssq = sp.tile([128, 32], F32, tag="ssq")
    nc.vector.tensor_reduce(
        ssq[:, :], sq[:, :].rearrange("p (t j) -> p j t", t=16),
        mybir.AxisListType.X, AluOp.add)
    t1 = sp.tile([128, 32], F32, tag="t1")
    nc.vector.tensor_scalar_add(t1[:, :], ssq[:, :], 1.0)
    r1 = sp.tile([128, 32], F32, tag="r1")
    nc.vector.reciprocal(r1[:, :], t1[:, :])
    ssqe = sp.tile([128, 32], F32, tag="ssqe")
    nc.vector.tensor_scalar_add(ssqe[:, :], ssq[:, :], EPS)
    t2 = sp.tile([128, 32], F32, tag="t2")
    nc.scalar.activation(t2[:, :], ssqe[:, :], Act.Sqrt, bias=0.0)
    r2 = sp.tile([128, 32], F32, tag="r2")
    nc.vector.reciprocal(r2[:, :], t2[:, :])
    sc = sp.tile([128, 32], F32, tag="sc")
    nc.vector.tensor_mul(sc[:, :], r1[:, :], r2[:, :])
    nc.vector.tensor_mul(sc[:, :], sc[:, :], ssq[:, :])
    scb = sp.tile([128, 32], BF16, tag="scb")
    nc.vector.tensor_copy(scb[:, :], sc[:, :])
    vexp = sp.tile([128, 512], BF16, tag=f"vexp{r}")
    nc.vector.tensor_mul(
        vexp[:, :].rearrange("p (t j) -> p t j", t=16),
        s128[:, :].rearrange("p (t j) -> p t j", t=16),
        scb[:, :].unsqueeze(1).to_broadcast([128, 16, 32]))
    if out_f32 is not None:
        vf = sp.tile([32, 512], F32, tag="vf")
        nc.vector.tensor_mul(
            vf[:, :].rearrange("p (t j) -> p t j", t=16),
            s128[0:32, :].rearrange("p (t j) -> p t j", t=16),
            scb[0:32, :].unsqueeze(1).to_broadcast([32, 16, 32]))
        nc.sync.dma_start(out_f32, vf[:, :])
    return vexp


@functools.lru_cache(maxsize=2)
def _get_nc(single=False):
    return _build_program(single=single)


def _prep_inputs(inputs, W):
    """Build per-core input maps (host-side layout only)."""
    inputs = np.asarray(inputs, dtype=np.float32)
    W = np.asarray(W, dtype=np.float32)
    W0 = W[0]  # [J, I, T, D]

    # delta_b ones [K=(i_sub 4, b 32), M=(b' 32)]
    ones = np.zeros((4, 32, 32), dtype=np.float32)
    for b in range(32):
        ones[:, b, b] = 1.0
    ones = ones.reshape(128, 32).astype(ml_dtypes.bfloat16)

    in_maps = []
    for c in range(NCORES):
        isl = slice(c * ILOC, (c + 1) * ILOC)
        ws = W0[:, isl]  # [J, 256, T, D]
        # wt[p, (gl, i_sub, d), (j, t)] ; i = (2p+gl)*4 + i_sub
        A = ws.transpose(1, 3, 0, 2)  # [i, d, j, t]
        A = A.reshape(NWT, 2, 4, D, J, T)  # p, gl, i_sub, d, j, t
        wtc = A.reshape(NWT, 128, J * T).astype(ml_dtypes.bfloat16)
        wtc = np.ascontiguousarray(
            wtc.reshape(NWG, 4, 128, 512).transpose(0, 2, 1, 3).reshape(
                NWG, 128, 4 * 512))

        xs = inputs[:, isl]  # [b, 256, d]
        xt = xs.transpose(1, 2, 0)  # [i, d, b]
        xt = xt.reshape(NWT, 2, 4, D, B)  # p, gl, i_sub, d, b
        # dense stationary for s0-direct: xs8[(gl,i_sub,d), (p, b)]
        xs8 = np.ascontiguousarray(
            xt.reshape(NWT, 128, B).transpose(1, 0, 2).reshape(128, NWT * B)
        ).astype(ml_dtypes.bfloat16)
        # block-diag stationary for phase A
        xdc = np.zeros((NWT, 2, 4, D, 4, B), dtype=np.float32)
        ar = np.arange(4)
        xdc[:, :, ar, :, ar, :] = xt.transpose(2, 0, 1, 3, 4)
        xdc = xdc.reshape(NWT, 2, 64, 128)  # p, gl, (i_sub d), (i_sub' b)
        xallc = np.ascontiguousarray(
            xdc.transpose(1, 2, 0, 3).reshape(128, NWT * 128)
        ).astype(ml_dtypes.bfloat16)

        in_maps.append({"wt": wtc, "xall": xallc, "xs8": xs8, "onesdb": ones})
    return in_maps


def _post(res):
    v = np.asarray(res.results[0]["outv"])  # [32, 512] = [b, (t, j)]
    return np.ascontiguousarray(
        v.reshape(B, T, J).transpose(0, 2, 1)
    ).astype(np.float32)


def kernel(inputs, W):
    import concourse.bass_utils as bass_utils

    nc = _get_nc()
    in_maps = _prep_inputs(inputs, W)
    res = bass_utils.run_bass_kernel_spmd(nc, in_maps, list(range(NCORES)))
    return _post(res)
